# revision 47
# baseline (speedup 1.0000x reference)
"""Trainium2 Bass kernel for nn_Discriminator (MLP + BN + attn + minibatch discrimination).

Strategy (8 NeuronCores, no collectives):
  - Shard the O(B^2) MBD block over the output index j: core d computes scores for
    batch rows [128d, 128d+128). SPMD programs are identical; the shard is selected
    by giving core d a batch-rolled copy of x (np.roll by -128d), so "my j's" are
    always local rows 0..127 while the i-sum still runs over the full batch.
  - Each core runs the full (tiny) MLP in transposed layout (features on partitions,
    batch on free axis), producing M^T [250, 1024] = (h @ T.reshape(64,250))^T.
  - Pairwise block per j: one dual-op tensor_scalar per 125-row fk-tile computes
    A = |M^T - M^T[:, j]| (bf16), a 0/1 selection-matrix matmul on the PE sums over
    k (5) into PSUM d = sum_k A [50 f, 1024 i] (two j's packed at PSUM partition
    offsets 0 and 64 via col-tiling), and one activation(Exp, scale=-1,
    accum_out=...) computes exp(-d) and the i-sum in a single ACT op.
  - score = Ws_h.T h + Ws_o.T o + (bs - sum(Ws_o)), the bias fold absorbing the
    reference's "-1" self-term removal.
"""

import numpy as np
from contextlib import ExitStack

import ml_dtypes
import concourse.bass as bass
import concourse.tile as tile
from concourse import mybir
from concourse.bass_utils import run_bass_kernel_spmd

F32 = mybir.dt.float32
BF16 = mybir.dt.bfloat16
AF = mybir.ActivationFunctionType
ALU = mybir.AluOpType
AX = mybir.AxisListType

B = 1024
IN_DIM = 128
NCORES = 8
JSH = B // NCORES          # 128 j's per core
NPAIR = JSH // 2           # 64 pairs of j's
FK = 250                   # 50 features x 5 kernels
FKH = 125                  # fk half-tile (f 0..24 | f 25..49)
NF = 50
BN_EPS = 1e-5

_CACHE: dict = {}


def _emit_body(tc, d, score_out):
    nc = tc.nc
    ctx = ExitStack()
    with ctx:
        consts = ctx.enter_context(tc.tile_pool(name="consts", bufs=1))
        mlp = ctx.enter_context(tc.tile_pool(name="mlp", bufs=1))
        small = ctx.enter_context(tc.tile_pool(name="small", bufs=1))

        # ---- load constants ----
        def cload(name, shape, dtype=F32):
            t = consts.tile(shape, dtype, tag=name)
            nc.sync.dma_start(t[:], d[name][:])
            return t

        # all f32 constants ride in one packed DMA; bf16 in a second
        CPF = consts.tile([128, 1015], F32, tag="CPF")
        nc.sync.dma_start(CPF[:], d["CPF"][:])
        CPB = consts.tile([125, 256], BF16, tag="CPB")
        nc.sync.dma_start(CPB[:], d["CPB"][:])
        W1 = CPF[:, 0:256]
        W2a = CPF[:, 256:384]
        W2b = CPF[:, 384:512]
        W3 = CPF[:, 512:576]
        Wv = CPF[0:64, 576:640]
        Wo = CPF[0:64, 640:704]
        Tm = CPF[0:64, 704:954]
        TmSn = CPF[0:64, 954:1004]
        b1a = CPF[:, 1004:1005]
        b1b = CPF[:, 1005:1006]
        b2 = CPF[:, 1006:1007]
        gamma = CPF[:, 1007:1008]
        beta = CPF[:, 1008:1009]
        b3 = CPF[0:64, 1009:1010]
        bv = CPF[0:64, 1010:1011]
        bo = CPF[0:64, 1011:1012]
        WsH = CPF[0:64, 1012:1013]
        WsO = CPF[0:50, 1013:1014]
        bsf = CPF[0:1, 1014:1015]
        Sa = CPB[:, 0:64]
        Sb = CPB[:, 64:128]
        I50h2 = CPB[0:50, 128:256]

        # ---- persistent activations (feature-major) ----
        xT = mlp.tile([128, B], F32, tag="xT")
        h1T = mlp.tile([128, 2 * B], F32, tag="h1T")      # [256,1024] as 2 M-tiles
        hbnT = mlp.tile([128, B], F32, tag="hbnT")
        h3T = mlp.tile([64, B], F32, tag="h3T")
        uT = mlp.tile([64, B], F32, tag="uT")
        hT = mlp.tile([64, B], F32, tag="hT")
        MTf = mlp.tile([125, 2 * B], F32, tag="MTf")      # [250,1024] as 2 fk-tiles
        MTb = mlp.tile([125, 2 * B], BF16, tag="MTb")
        OBUF = mlp.tile([128, NPAIR], F32, tag="OBUF")
        O50 = mlp.tile([50, NPAIR, 2], F32, tag="O50")
        SMTn = mlp.tile([50, B], F32, tag="SMTn")
        SMTnb = mlp.tile([50, B], BF16, tag="SMTnb")
        BIASP = mlp.tile([128, NPAIR], F32, tag="BIASP")

        def lrelu(dst, src):
            # dst = max(src, 0.2*src)
            nc.vector.scalar_tensor_tensor(
                out=dst, in0=src, scalar=0.2, in1=src, op0=ALU.mult, op1=ALU.max
            )

        with tc.tile_pool(name="ph1_psum", bufs=1, space=bass.MemorySpace.PSUM) as pp, \
             tc.tile_pool(name="ph1_sb", bufs=4) as sb:
            # ---- xT loaded directly (host pre-transposes x) ----
            nc.sync.dma_start(xT[:], d["xT"][:])

            # ---- h1T = lrelu(W1.T xT + b1) ----
            for mt, b1t in ((0, b1a), (1, b1b)):
                for c in range(2):
                    ps = pp.tile([128, 512], F32, tag="ps", bufs=2)
                    nc.tensor.matmul(ps[:], W1[:, 128 * mt:128 * (mt + 1)],
                                     xT[:, 512 * c:512 * (c + 1)], start=True, stop=True)
                    tt = sb.tile([128, 512], F32, tag="tt")
                    nc.scalar.activation(tt[:], ps[:], AF.Identity, bias=b1t, scale=1.0)
                    lrelu(h1T[:, B * mt + 512 * c: B * mt + 512 * (c + 1)], tt[:])

            # ---- h2 (kept in PSUM) + BN stats ----
            h2ps = []
            sums = small.tile([128, 4], F32, tag="sums")   # per-chunk sum, sumsq
            for c in range(2):
                ps = pp.tile([128, 512], F32, tag=f"h2ps{c}")
                for kt, W2t in ((0, W2a), (1, W2b)):
                    nc.tensor.matmul(ps[:], W2t[:],
                                     h1T[:, B * kt + 512 * c: B * kt + 512 * (c + 1)],
                                     start=(kt == 0), stop=(kt == 1))
                # bias b2 folds into BN shift below (h2+b2 then BN). Since BN
                # subtracts the batch mean, adding b2 cancels: (h+b2) - mean(h+b2)
                # = h - mean(h). Variance likewise unaffected. So skip b2 here.
                nc.vector.tensor_reduce(sums[:, c:c + 1], ps[:], axis=AX.X, op=ALU.add)
                sq = sb.tile([128, 512], F32, tag="sq")
                nc.scalar.activation(sq[:], ps[:], AF.Square, bias=0.0, scale=1.0,
                                     accum_out=sums[:, 2 + c:3 + c])
                h2ps.append(ps)

            # mu = (s0+s1)/1024 ; msq = (q0+q1)/1024 ; var = msq - mu^2
            mu = small.tile([128, 1], F32, tag="mu")
            nc.vector.scalar_tensor_tensor(out=mu[:], in0=sums[:, 0:1], scalar=1.0 / B,
                                           in1=sums[:, 1:2], op0=ALU.bypass, op1=ALU.add)
            nc.vector.tensor_scalar(out=mu[:], in0=mu[:], scalar1=1.0 / B, scalar2=None,
                                    op0=ALU.mult)
            msq = small.tile([128, 1], F32, tag="msq")
            nc.vector.scalar_tensor_tensor(out=msq[:], in0=sums[:, 2:3], scalar=1.0,
                                           in1=sums[:, 3:4], op0=ALU.bypass, op1=ALU.add)
            nc.vector.tensor_scalar(out=msq[:], in0=msq[:], scalar1=1.0 / B, scalar2=None,
                                    op0=ALU.mult)
            var = small.tile([128, 1], F32, tag="var")
            nc.vector.scalar_tensor_tensor(out=var[:], in0=mu[:], scalar=-1.0,
                                           in1=mu[:], op0=ALU.mult, op1=ALU.mult)
            nc.vector.tensor_tensor(out=var[:], in0=var[:], in1=msq[:], op=ALU.add)
            # invstd = exp(-0.5*ln(var+eps))  (avoids the banned Rsqrt and the sqrt table set)
            eps_t = small.tile([128, 1], F32, tag="eps")
            nc.vector.memset(eps_t[:], BN_EPS)
            lnv = small.tile([128, 1], F32, tag="lnv")
            nc.scalar.activation(lnv[:], var[:], AF.Ln, bias=eps_t[:], scale=1.0)
            invstd = small.tile([128, 1], F32, tag="invstd")
            nc.scalar.activation(invstd[:], lnv[:], AF.Exp, bias=0.0, scale=-0.5)
            # s = gamma*invstd ; bb = beta - mu*s  (+ b2 folded: cancels, see above)
            s = small.tile([128, 1], F32, tag="s")
            nc.vector.tensor_tensor(out=s[:], in0=invstd[:], in1=gamma[:], op=ALU.mult)
            bb = small.tile([128, 1], F32, tag="bb")
            nc.vector.scalar_tensor_tensor(out=bb[:], in0=mu[:], scalar=-1.0,
                                           in1=s[:], op0=ALU.mult, op1=ALU.mult)
            nc.vector.tensor_tensor(out=bb[:], in0=bb[:], in1=beta[:], op=ALU.add)

            # hbnT = lrelu(s*h2 + bb)
            for c in range(2):
                tt = sb.tile([128, 512], F32, tag="tt")
                nc.scalar.activation(tt[:], h2ps[c][:], AF.Identity, bias=bb[:, 0:1],
                                     scale=s[:, 0:1])
                lrelu(hbnT[:, 512 * c:512 * (c + 1)], tt[:])

            # ---- h3T = lrelu(W3.T hbnT + b3) ----
            for c in range(2):
                ps = pp.tile([64, 512], F32, tag="ps64", bufs=2)
                nc.tensor.matmul(ps[:], W3[:], hbnT[:, 512 * c:512 * (c + 1)],
                                 start=True, stop=True)
                tt = sb.tile([64, 512], F32, tag="tt64")
                nc.scalar.activation(tt[:], ps[:], AF.Identity, bias=b3, scale=1.0)
                lrelu(h3T[:, 512 * c:512 * (c + 1)], tt[:])

            # ---- uT = Wv.T h3T + bv ----
            for c in range(2):
                ps = pp.tile([64, 512], F32, tag="ps64", bufs=2)
                nc.tensor.matmul(ps[:], Wv[:], h3T[:, 512 * c:512 * (c + 1)],
                                 start=True, stop=True)
                nc.scalar.activation(uT[:, 512 * c:512 * (c + 1)], ps[:], AF.Identity,
                                     bias=bv, scale=1.0)

            # ---- hT = h3T + Wo.T uT + bo ----
            for c in range(2):
                ps = pp.tile([64, 512], F32, tag="ps64", bufs=2)
                nc.tensor.matmul(ps[:], Wo[:], uT[:, 512 * c:512 * (c + 1)],
                                 start=True, stop=True)
                nc.vector.scalar_tensor_tensor(
                    out=hT[:, 512 * c:512 * (c + 1)], in0=ps[:], scalar=bo,
                    in1=h3T[:, 512 * c:512 * (c + 1)], op0=ALU.add, op1=ALU.add)

            # ---- MT = Tm.T hT  ([250,1024] as 2 fk-tiles), f32 + bf16 copies ----
            for st in range(2):
                for c in range(2):
                    ps = pp.tile([125, 512], F32, tag="psm", bufs=1)
                    nc.tensor.matmul(ps[:], Tm[:, 125 * st:125 * (st + 1)],
                                     hT[:, 512 * c:512 * (c + 1)], start=True, stop=True)
                    sl = slice(B * st + 512 * c, B * st + 512 * (c + 1))
                    nc.vector.tensor_copy(MTf[:, sl], ps[:])
                    nc.scalar.activation(MTb[:, sl], ps[:], AF.Copy, bias=0.0, scale=1.0)

            # ---- SMTn[f, i] = -sum_k M[i, 5f+k]  (for the |d|=2relu(d)-d trick) ----
            for c in range(2):
                ps = pp.tile([50, 512], F32, tag="psm2", bufs=1)
                nc.tensor.matmul(ps[:], TmSn[:], hT[:, 512 * c:512 * (c + 1)],
                                 start=True, stop=True)
                nc.vector.tensor_copy(SMTn[:, 512 * c:512 * (c + 1)], ps[:])
                nc.scalar.activation(SMTnb[:, 512 * c:512 * (c + 1)], ps[:], AF.Copy,
                                     bias=0.0, scale=1.0)

            # per-pair exp bias rows: [0:50] <- SMTn col j1, [64:114] <- SMTn col j2
            nc.vector.memset(BIASP[:], 0.0)
            nc.vector.tensor_copy(BIASP[0:50, :], SMTn[:, 0:JSH].rearrange(
                "p (a b) -> p a b", b=2)[:, :, 0:1])
            nc.vector.tensor_copy(BIASP[64:114, :], SMTn[:, 0:JSH].rearrange(
                "p (a b) -> p a b", b=2)[:, :, 1:2])

        # ---- pairwise MBD block ----
        # d[f,i] for row j is sum_k |M_i - M_j| = 2*sum_k relu(M_i - M_j)
        #   - sum_k M_i + sum_k M_j.  PSUM accumulates P = SAp + 0.5*SMTn_i;
        # exp(-d) = Exp(-2*P + bias) with per-partition bias = SMTn[:, j].
        with tc.tile_pool(name="apool", bufs=12) as apool, \
             tc.tile_pool(name="dpool", bufs=4, space=bass.MemorySpace.PSUM) as dpool:
            for jp in range(NPAIR):
                j1, j2 = 2 * jp, 2 * jp + 1
                As = {}
                for (jj, col) in ((j1, 0), (j2, 64)):
                    for st in range(2):
                        A = apool.tile([125, B], BF16, tag=f"A{col}{st}")
                        nc.vector.tensor_scalar(
                            out=A[:], in0=MTb[:, B * st:B * (st + 1)],
                            scalar1=MTf[:, B * st + jj:B * st + jj + 1],
                            scalar2=0.0, op0=ALU.subtract, op1=ALU.max)
                        As[(col, st)] = A
                dps = dpool.tile([128, B], F32, tag="dps")
                # The SMT-correction matmul only needs phase-1 data, so it
                # OPENS the accumulation group (start=True): the scheduler can
                # run it before this pair's A tiles exist, filling PE idle at
                # the phase transition with useful work. Sa/Sb follow,
                # col-group-interleaved for array concurrency.
                for c in range(2):
                    cs = slice(512 * c, 512 * (c + 1))
                    nc.tensor.matmul(dps[0:128, cs], I50h2[:], SMTnb[:, cs],
                                     start=True, stop=False,
                                     skip_group_check=True)
                for st, S in ((0, Sa), (1, Sb)):
                    for c in range(2):
                        cs = slice(512 * c, 512 * (c + 1))
                        for col in (0, 64):
                            nc.tensor.matmul(dps[col:col + 64, cs], S[:],
                                             As[(col, st)][:, cs],
                                             start=False, stop=(st == 1),
                                             tile_position=(0, col),
                                             skip_group_check=True)
                nc.scalar.activation(dps[0:114, :], dps[0:114, :], AF.Exp,
                                     bias=BIASP[0:114, jp:jp + 1], scale=-2.0,
                                     accum_out=OBUF[0:114, jp:jp + 1])

            # ---- o columns -> j-ordered [50, 128] ----
            nc.vector.tensor_copy(O50[:, :, 0:1], OBUF[0:50, :])
            nc.vector.tensor_copy(O50[:, :, 1:2], OBUF[64:114, :])

        # ---- score = WsH.T hT[:, :128] + WsO.T O + bsf (pairwise pools freed) ----
        with tc.tile_pool(name="spsum", bufs=1, space=bass.MemorySpace.PSUM) as sp:
            ssum = sp.tile([1, JSH], F32, tag="ssum")
            nc.tensor.matmul(ssum[:], WsH[:], hT[:, 0:JSH], start=True, stop=False)
            nc.tensor.matmul(ssum[:], WsO[:], O50[:, :, :], start=False, stop=True)
            sc = small.tile([1, JSH], F32, tag="sc")
            nc.scalar.activation(sc[:], ssum[:], AF.Identity, bias=bsf[0:1, 0:1],
                                 scale=1.0)
            nc.gpsimd.dma_start(score_out[:], sc[:])


def _split_waits(nc):
    """Hoist excess semaphore waits onto single-wait engine nops.

    This walrus build's codegen rejects instructions whose ISA struct carries
    more than one sync-wait ("Too many sync wait commands", e.g. the
    self-loading fp32 LDW+MM path). Engine instruction streams execute in
    order, so moving all waits of an instruction onto nop instructions spliced
    immediately before it (one wait per nop, same engine) is semantically
    identical. DMA instructions are left untouched (their waits ride the DGE
    descriptor, not the engine stream) and are asserted to have <=1 wait.
    """
    from concourse import mybir as mb
    DMA_TYPES = (mb.InstDMACopy, mb.InstDMA, mb.InstTriggeredCopy) \
        if hasattr(mb, "InstTriggeredCopy") else (mb.InstDMACopy, mb.InstDMA)
    for fn in nc.m.functions:
        for bb in fn.blocks:
            insts = list(bb.instructions)
            out = []
            for inst in insts:
                si = inst.sync_info
                waits = list(si.on_wait) if si is not None else []
                if len(waits) > 1:
                    if isinstance(inst, DMA_TYPES):
                        raise AssertionError(
                            f"DMA instruction {inst.name} has {len(waits)} waits; "
                            "cannot split safely — restructure the kernel")
                    for w in waits:
                        nop = mb.InstNoOp(
                            name=nc.get_next_instruction_name(),
                            ins=[], outs=[])
                        nop.engine = inst.engine
                        nop.sync_info = mb.SyncInfo(on_wait=[w], on_update=[])
                        nc.register_instruction(nop)
                        out.append(nop)
                    inst.sync_info = mb.SyncInfo(
                        on_wait=[], on_update=list(si.on_update))
                out.append(inst)
            bb.instructions = out


def _build():
    nc = bass.Bass("TRN2", target_bir_lowering=False, debug=False,
                   num_devices=NCORES)
    d = {}

    def din(name, shape, dtype=F32):
        d[name] = nc.dram_tensor(name, shape, dtype, kind="ExternalInput").ap()

    din("xT", [IN_DIM, B])
    din("CPF", [128, 1015])
    din("CPB", [125, 256], BF16)
    score = nc.dram_tensor("score", [1, JSH], F32, kind="ExternalOutput").ap()

    with tile.TileContext(nc) as tc:
        _emit_body(tc, d, score)
    _split_waits(nc)
    return nc


def get_nc():
    if "nc" not in _CACHE:
        _CACHE["nc"] = _build()
    return _CACHE["nc"]


def _make_in_maps(inputs):
    f = lambda a: np.ascontiguousarray(np.asarray(a, dtype=np.float32))
    x = f(inputs["x"])
    Tm = f(inputs["T"]).reshape(64, 250)
    Sa = np.zeros((125, 64), np.float32)
    Sb = np.zeros((125, 64), np.float32)
    for fk in range(125):
        Sa[fk, fk // 5] = 1.0
        Sb[fk, 25 + fk // 5] = 1.0
    TmS = Tm.reshape(64, 50, 5).sum(axis=2)
    TmSn = np.ascontiguousarray(-TmS)
    I50h2 = np.zeros((50, 128), np.float32)
    np.fill_diagonal(I50h2[:, 0:50], 0.5)
    np.fill_diagonal(I50h2[:, 64:114], 0.5)
    Ws = f(inputs["Ws"])
    bsf = np.array([[float(f(inputs["bs"]).reshape(-1)[0]) - float(Ws[64:].sum())]],
                   np.float32)
    CPF = np.zeros((128, 1015), np.float32)
    CPF[:, 0:256] = f(inputs["W1"])
    CPF[:, 256:384] = f(inputs["W2"])[0:128]
    CPF[:, 384:512] = f(inputs["W2"])[128:256]
    CPF[:, 512:576] = f(inputs["W3"])
    CPF[0:64, 576:640] = f(inputs["Wv"])
    CPF[0:64, 640:704] = f(inputs["Wo"])
    CPF[0:64, 704:954] = Tm
    CPF[0:64, 954:1004] = TmSn
    CPF[:, 1004] = f(inputs["b1"]).reshape(-1)[0:128]
    CPF[:, 1005] = f(inputs["b1"]).reshape(-1)[128:256]
    CPF[:, 1006] = f(inputs["b2"]).reshape(-1)
    CPF[:, 1007] = f(inputs["gamma"]).reshape(-1)
    CPF[:, 1008] = f(inputs["beta"]).reshape(-1)
    CPF[0:64, 1009] = f(inputs["b3"]).reshape(-1)
    CPF[0:64, 1010] = f(inputs["bv"]).reshape(-1)
    CPF[0:64, 1011] = f(inputs["bo"]).reshape(-1)
    CPF[0:64, 1012] = Ws[:64, 0]
    CPF[0:50, 1013] = Ws[64:, 0]
    CPF[0, 1014] = bsf[0, 0]
    CPB = np.zeros((125, 256), np.float32)
    CPB[:, 0:64] = Sa
    CPB[:, 64:128] = Sb
    CPB[0:50, 128:256] = I50h2
    common = {
        "CPF": CPF,
        "CPB": CPB.astype(ml_dtypes.bfloat16),
    }
    in_maps = []
    for c in range(NCORES):
        m = dict(common)
        m["xT"] = np.ascontiguousarray(np.roll(x, -JSH * c, axis=0).T)
        in_maps.append(m)
    return in_maps


def kernel(**inputs) -> np.ndarray:
    nc = get_nc()
    in_maps = _make_in_maps(inputs)
    res = run_bass_kernel_spmd(nc, in_maps, list(range(NCORES)))
    outs = [np.asarray(res.results[c]["score"]).reshape(JSH) for c in range(NCORES)]
    return np.concatenate(outs).astype(np.float32)


if __name__ == "__main__":
    print("building nc...")
    nc = get_nc()
    print("build OK")


# revision 48
# speedup vs baseline: 1.0004x; 1.0004x over previous
"""Trainium2 Bass kernel for nn_Discriminator (MLP + BN + attn + minibatch discrimination).

Strategy (8 NeuronCores, no collectives):
  - Shard the O(B^2) MBD block over the output index j: core d computes scores for
    batch rows [128d, 128d+128). SPMD programs are identical; the shard is selected
    by giving core d a batch-rolled copy of x (np.roll by -128d), so "my j's" are
    always local rows 0..127 while the i-sum still runs over the full batch.
  - Each core runs the full (tiny) MLP in transposed layout (features on partitions,
    batch on free axis), producing M^T [250, 1024] = (h @ T.reshape(64,250))^T.
  - Pairwise block per j: one dual-op tensor_scalar per 125-row fk-tile computes
    A = |M^T - M^T[:, j]| (bf16), a 0/1 selection-matrix matmul on the PE sums over
    k (5) into PSUM d = sum_k A [50 f, 1024 i] (two j's packed at PSUM partition
    offsets 0 and 64 via col-tiling), and one activation(Exp, scale=-1,
    accum_out=...) computes exp(-d) and the i-sum in a single ACT op.
  - score = Ws_h.T h + Ws_o.T o + (bs - sum(Ws_o)), the bias fold absorbing the
    reference's "-1" self-term removal.
"""

import numpy as np
from contextlib import ExitStack

import ml_dtypes
import concourse.bass as bass
import concourse.tile as tile
from concourse import mybir
from concourse.bass_utils import run_bass_kernel_spmd

F32 = mybir.dt.float32
BF16 = mybir.dt.bfloat16
AF = mybir.ActivationFunctionType
ALU = mybir.AluOpType
AX = mybir.AxisListType

B = 1024
IN_DIM = 128
NCORES = 8
JSH = B // NCORES          # 128 j's per core
NPAIR = JSH // 2           # 64 pairs of j's
FK = 250                   # 50 features x 5 kernels
FKH = 125                  # fk half-tile (f 0..24 | f 25..49)
NF = 50
BN_EPS = 1e-5

_CACHE: dict = {}


def _emit_body(tc, d, score_out):
    nc = tc.nc
    ctx = ExitStack()
    with ctx:
        consts = ctx.enter_context(tc.tile_pool(name="consts", bufs=1))
        mlp = ctx.enter_context(tc.tile_pool(name="mlp", bufs=1))
        small = ctx.enter_context(tc.tile_pool(name="small", bufs=1))

        # ---- load constants ----
        def cload(name, shape, dtype=F32):
            t = consts.tile(shape, dtype, tag=name)
            nc.sync.dma_start(t[:], d[name][:])
            return t

        # all f32 constants ride in one packed DMA; bf16 in a second
        CPF = consts.tile([128, 1015], F32, tag="CPF")
        nc.sync.dma_start(CPF[:], d["CPF"][:])
        CPB = consts.tile([125, 256], BF16, tag="CPB")
        nc.sync.dma_start(CPB[:], d["CPB"][:])
        W1 = CPF[:, 0:256]
        W2a = CPF[:, 256:384]
        W2b = CPF[:, 384:512]
        W3 = CPF[:, 512:576]
        Wv = CPF[0:64, 576:640]
        Wo = CPF[0:64, 640:704]
        Tm = CPF[0:64, 704:954]
        TmSn = CPF[0:64, 954:1004]
        b1a = CPF[:, 1004:1005]
        b1b = CPF[:, 1005:1006]
        b2 = CPF[:, 1006:1007]
        gamma = CPF[:, 1007:1008]
        beta = CPF[:, 1008:1009]
        b3 = CPF[0:64, 1009:1010]
        bv = CPF[0:64, 1010:1011]
        bo = CPF[0:64, 1011:1012]
        WsH = CPF[0:64, 1012:1013]
        WsO = CPF[0:50, 1013:1014]
        bsf = CPF[0:1, 1014:1015]
        Sa = CPB[:, 0:64]
        Sb = CPB[:, 64:128]
        I50h2 = CPB[0:50, 128:256]

        # ---- persistent activations (feature-major) ----
        xT = mlp.tile([128, B], F32, tag="xT")
        h1T = mlp.tile([128, 2 * B], F32, tag="h1T")      # [256,1024] as 2 M-tiles
        hbnT = mlp.tile([128, B], F32, tag="hbnT")
        h3T = mlp.tile([64, B], F32, tag="h3T")
        uT = mlp.tile([64, B], F32, tag="uT")
        hT = mlp.tile([64, B], F32, tag="hT")
        MTf = mlp.tile([125, 2 * B], F32, tag="MTf")      # [250,1024] as 2 fk-tiles
        MTb = mlp.tile([125, 2 * B], BF16, tag="MTb")
        OBUF = mlp.tile([128, NPAIR], F32, tag="OBUF")
        O50 = mlp.tile([50, NPAIR, 2], F32, tag="O50")
        SMTn = mlp.tile([50, B], F32, tag="SMTn")
        SMTnb = mlp.tile([50, B], BF16, tag="SMTnb")
        BIASP = mlp.tile([128, NPAIR], F32, tag="BIASP")

        def lrelu(dst, src):
            # dst = max(src, 0.2*src)
            nc.vector.scalar_tensor_tensor(
                out=dst, in0=src, scalar=0.2, in1=src, op0=ALU.mult, op1=ALU.max
            )

        with tc.tile_pool(name="ph1_psum", bufs=1, space=bass.MemorySpace.PSUM) as pp, \
             tc.tile_pool(name="ph1_sb", bufs=4) as sb:
            # ---- xT loaded directly (host pre-transposes x) ----
            nc.sync.dma_start(xT[:], d["xT"][:])

            # ---- h1T = lrelu(W1.T xT + b1) ----
            for mt, b1t in ((0, b1a), (1, b1b)):
                for c in range(2):
                    ps = pp.tile([128, 512], F32, tag="ps", bufs=2)
                    nc.tensor.matmul(ps[:], W1[:, 128 * mt:128 * (mt + 1)],
                                     xT[:, 512 * c:512 * (c + 1)], start=True, stop=True)
                    tt = sb.tile([128, 512], F32, tag="tt")
                    nc.scalar.activation(tt[:], ps[:], AF.Identity, bias=b1t, scale=1.0)
                    lrelu(h1T[:, B * mt + 512 * c: B * mt + 512 * (c + 1)], tt[:])

            # ---- h2 (kept in PSUM) + BN stats ----
            h2ps = []
            sums = small.tile([128, 4], F32, tag="sums")   # per-chunk sum, sumsq
            for c in range(2):
                ps = pp.tile([128, 512], F32, tag=f"h2ps{c}")
                for kt, W2t in ((0, W2a), (1, W2b)):
                    nc.tensor.matmul(ps[:], W2t[:],
                                     h1T[:, B * kt + 512 * c: B * kt + 512 * (c + 1)],
                                     start=(kt == 0), stop=(kt == 1))
                # bias b2 folds into BN shift below (h2+b2 then BN). Since BN
                # subtracts the batch mean, adding b2 cancels: (h+b2) - mean(h+b2)
                # = h - mean(h). Variance likewise unaffected. So skip b2 here.
                nc.vector.tensor_reduce(sums[:, c:c + 1], ps[:], axis=AX.X, op=ALU.add)
                sq = sb.tile([128, 512], F32, tag="sq")
                nc.scalar.activation(sq[:], ps[:], AF.Square, bias=0.0, scale=1.0,
                                     accum_out=sums[:, 2 + c:3 + c])
                h2ps.append(ps)

            # mu = (s0+s1)/1024 ; msq = (q0+q1)/1024 ; var = msq - mu^2
            mu = small.tile([128, 1], F32, tag="mu")
            nc.vector.scalar_tensor_tensor(out=mu[:], in0=sums[:, 0:1], scalar=1.0 / B,
                                           in1=sums[:, 1:2], op0=ALU.bypass, op1=ALU.add)
            nc.vector.tensor_scalar(out=mu[:], in0=mu[:], scalar1=1.0 / B, scalar2=None,
                                    op0=ALU.mult)
            msq = small.tile([128, 1], F32, tag="msq")
            nc.vector.scalar_tensor_tensor(out=msq[:], in0=sums[:, 2:3], scalar=1.0,
                                           in1=sums[:, 3:4], op0=ALU.bypass, op1=ALU.add)
            nc.vector.tensor_scalar(out=msq[:], in0=msq[:], scalar1=1.0 / B, scalar2=None,
                                    op0=ALU.mult)
            var = small.tile([128, 1], F32, tag="var")
            nc.vector.scalar_tensor_tensor(out=var[:], in0=mu[:], scalar=-1.0,
                                           in1=mu[:], op0=ALU.mult, op1=ALU.mult)
            nc.vector.tensor_tensor(out=var[:], in0=var[:], in1=msq[:], op=ALU.add)
            # invstd = exp(-0.5*ln(var+eps))  (avoids the banned Rsqrt and the sqrt table set)
            eps_t = small.tile([128, 1], F32, tag="eps")
            nc.vector.memset(eps_t[:], BN_EPS)
            lnv = small.tile([128, 1], F32, tag="lnv")
            nc.scalar.activation(lnv[:], var[:], AF.Ln, bias=eps_t[:], scale=1.0)
            invstd = small.tile([128, 1], F32, tag="invstd")
            nc.scalar.activation(invstd[:], lnv[:], AF.Exp, bias=0.0, scale=-0.5)
            # s = gamma*invstd ; bb = beta - mu*s  (+ b2 folded: cancels, see above)
            s = small.tile([128, 1], F32, tag="s")
            nc.vector.tensor_tensor(out=s[:], in0=invstd[:], in1=gamma[:], op=ALU.mult)
            bb = small.tile([128, 1], F32, tag="bb")
            nc.vector.scalar_tensor_tensor(out=bb[:], in0=mu[:], scalar=-1.0,
                                           in1=s[:], op0=ALU.mult, op1=ALU.mult)
            nc.vector.tensor_tensor(out=bb[:], in0=bb[:], in1=beta[:], op=ALU.add)

            # hbnT = lrelu(s*h2 + bb)
            for c in range(2):
                tt = sb.tile([128, 512], F32, tag="tt")
                nc.scalar.activation(tt[:], h2ps[c][:], AF.Identity, bias=bb[:, 0:1],
                                     scale=s[:, 0:1])
                lrelu(hbnT[:, 512 * c:512 * (c + 1)], tt[:])

            # ---- h3T = lrelu(W3.T hbnT + b3) ----
            for c in range(2):
                ps = pp.tile([64, 512], F32, tag="ps64", bufs=2)
                nc.tensor.matmul(ps[:], W3[:], hbnT[:, 512 * c:512 * (c + 1)],
                                 start=True, stop=True)
                tt = sb.tile([64, 512], F32, tag="tt64")
                nc.scalar.activation(tt[:], ps[:], AF.Identity, bias=b3, scale=1.0)
                lrelu(h3T[:, 512 * c:512 * (c + 1)], tt[:])

            # ---- uT = Wv.T h3T + bv ----
            for c in range(2):
                ps = pp.tile([64, 512], F32, tag="ps64", bufs=2)
                nc.tensor.matmul(ps[:], Wv[:], h3T[:, 512 * c:512 * (c + 1)],
                                 start=True, stop=True)
                nc.scalar.activation(uT[:, 512 * c:512 * (c + 1)], ps[:], AF.Identity,
                                     bias=bv, scale=1.0)

            # ---- hT = h3T + Wo.T uT + bo ----
            for c in range(2):
                ps = pp.tile([64, 512], F32, tag="ps64", bufs=2)
                nc.tensor.matmul(ps[:], Wo[:], uT[:, 512 * c:512 * (c + 1)],
                                 start=True, stop=True)
                nc.vector.scalar_tensor_tensor(
                    out=hT[:, 512 * c:512 * (c + 1)], in0=ps[:], scalar=bo,
                    in1=h3T[:, 512 * c:512 * (c + 1)], op0=ALU.add, op1=ALU.add)

            # ---- MT = Tm.T hT  ([250,1024] as 2 fk-tiles), f32 + bf16 copies ----
            for st in range(2):
                for c in range(2):
                    ps = pp.tile([125, 512], F32, tag="psm", bufs=1)
                    nc.tensor.matmul(ps[:], Tm[:, 125 * st:125 * (st + 1)],
                                     hT[:, 512 * c:512 * (c + 1)], start=True, stop=True)
                    sl = slice(B * st + 512 * c, B * st + 512 * (c + 1))
                    nc.vector.tensor_copy(MTf[:, sl], ps[:])
                    nc.scalar.activation(MTb[:, sl], ps[:], AF.Copy, bias=0.0, scale=1.0)

            # ---- SMTn[f, i] = -sum_k M[i, 5f+k]  (for the |d|=2relu(d)-d trick) ----
            for c in range(2):
                ps = pp.tile([50, 512], F32, tag="psm2", bufs=1)
                nc.tensor.matmul(ps[:], TmSn[:], hT[:, 512 * c:512 * (c + 1)],
                                 start=True, stop=True)
                nc.vector.tensor_copy(SMTn[:, 512 * c:512 * (c + 1)], ps[:])
                nc.scalar.activation(SMTnb[:, 512 * c:512 * (c + 1)], ps[:], AF.Copy,
                                     bias=0.0, scale=1.0)

            # per-pair exp bias rows: [0:50] <- SMTn col j1, [64:114] <- SMTn col j2
            nc.vector.memset(BIASP[:], 0.0)
            nc.vector.tensor_copy(BIASP[0:50, :], SMTn[:, 0:JSH].rearrange(
                "p (a b) -> p a b", b=2)[:, :, 0:1])
            nc.vector.tensor_copy(BIASP[64:114, :], SMTn[:, 0:JSH].rearrange(
                "p (a b) -> p a b", b=2)[:, :, 1:2])

        # ---- pairwise MBD block ----
        # d[f,i] for row j is sum_k |M_i - M_j| = 2*sum_k relu(M_i - M_j)
        #   - sum_k M_i + sum_k M_j.  PSUM accumulates P = SAp + 0.5*SMTn_i;
        # exp(-d) = Exp(-2*P + bias) with per-partition bias = SMTn[:, j].
        with tc.tile_pool(name="apool", bufs=12) as apool, \
             tc.tile_pool(name="dpool", bufs=4, space=bass.MemorySpace.PSUM) as dpool:
            for jp in range(NPAIR):
                j1, j2 = 2 * jp, 2 * jp + 1
                As = {}
                for (jj, col) in ((j1, 0), (j2, 64)):
                    for st in range(2):
                        A = apool.tile([125, B], BF16, tag=f"A{col}{st}")
                        nc.vector.tensor_scalar(
                            out=A[:], in0=MTb[:, B * st:B * (st + 1)],
                            scalar1=MTf[:, B * st + jj:B * st + jj + 1],
                            scalar2=0.0, op0=ALU.subtract, op1=ALU.max)
                        As[(col, st)] = A
                dps = dpool.tile([128, B], F32, tag="dps")
                # Sa/Sb: col-group-interleaved so adjacent MMs can run
                # concurrently in the array; the SMT correction rides ONE
                # full-width M=128 matmul (same rhs for both col groups)
                for st, S in ((0, Sa), (1, Sb)):
                    for c in range(2):
                        cs = slice(512 * c, 512 * (c + 1))
                        for col in (0, 64):
                            nc.tensor.matmul(dps[col:col + 64, cs], S[:],
                                             As[(col, st)][:, cs],
                                             start=(st == 0), stop=False,
                                             tile_position=(0, col),
                                             skip_group_check=True)
                for c in range(2):
                    cs = slice(512 * c, 512 * (c + 1))
                    nc.tensor.matmul(dps[0:128, cs], I50h2[:], SMTnb[:, cs],
                                     start=False, stop=True,
                                     skip_group_check=True)
                nc.scalar.activation(dps[0:114, :], dps[0:114, :], AF.Exp,
                                     bias=BIASP[0:114, jp:jp + 1], scale=-2.0,
                                     accum_out=OBUF[0:114, jp:jp + 1])

            # ---- o columns -> j-ordered [50, 128] ----
            nc.vector.tensor_copy(O50[:, :, 0:1], OBUF[0:50, :])
            nc.vector.tensor_copy(O50[:, :, 1:2], OBUF[64:114, :])

        # ---- score = WsH.T hT[:, :128] + WsO.T O + bsf (pairwise pools freed) ----
        with tc.tile_pool(name="spsum", bufs=1, space=bass.MemorySpace.PSUM) as sp:
            ssum = sp.tile([1, JSH], F32, tag="ssum")
            nc.tensor.matmul(ssum[:], WsH[:], hT[:, 0:JSH], start=True, stop=False)
            nc.tensor.matmul(ssum[:], WsO[:], O50[:, :, :], start=False, stop=True)
            sc = small.tile([1, JSH], F32, tag="sc")
            nc.scalar.activation(sc[:], ssum[:], AF.Identity, bias=bsf[0:1, 0:1],
                                 scale=1.0)
            nc.gpsimd.dma_start(score_out[:], sc[:])


def _split_waits(nc):
    """Hoist excess semaphore waits onto single-wait engine nops.

    This walrus build's codegen rejects instructions whose ISA struct carries
    more than one sync-wait ("Too many sync wait commands", e.g. the
    self-loading fp32 LDW+MM path). Engine instruction streams execute in
    order, so moving all waits of an instruction onto nop instructions spliced
    immediately before it (one wait per nop, same engine) is semantically
    identical. DMA instructions are left untouched (their waits ride the DGE
    descriptor, not the engine stream) and are asserted to have <=1 wait.
    """
    from concourse import mybir as mb
    DMA_TYPES = (mb.InstDMACopy, mb.InstDMA, mb.InstTriggeredCopy) \
        if hasattr(mb, "InstTriggeredCopy") else (mb.InstDMACopy, mb.InstDMA)
    for fn in nc.m.functions:
        for bb in fn.blocks:
            insts = list(bb.instructions)
            out = []
            for inst in insts:
                si = inst.sync_info
                waits = list(si.on_wait) if si is not None else []
                if len(waits) > 1:
                    if isinstance(inst, DMA_TYPES):
                        raise AssertionError(
                            f"DMA instruction {inst.name} has {len(waits)} waits; "
                            "cannot split safely — restructure the kernel")
                    for w in waits:
                        nop = mb.InstNoOp(
                            name=nc.get_next_instruction_name(),
                            ins=[], outs=[])
                        nop.engine = inst.engine
                        nop.sync_info = mb.SyncInfo(on_wait=[w], on_update=[])
                        nc.register_instruction(nop)
                        out.append(nop)
                    inst.sync_info = mb.SyncInfo(
                        on_wait=[], on_update=list(si.on_update))
                out.append(inst)
            bb.instructions = out


def _build():
    nc = bass.Bass("TRN2", target_bir_lowering=False, debug=False,
                   num_devices=NCORES)
    d = {}

    def din(name, shape, dtype=F32):
        d[name] = nc.dram_tensor(name, shape, dtype, kind="ExternalInput").ap()

    din("xT", [IN_DIM, B])
    din("CPF", [128, 1015])
    din("CPB", [125, 256], BF16)
    score = nc.dram_tensor("score", [1, JSH], F32, kind="ExternalOutput").ap()

    with tile.TileContext(nc) as tc:
        _emit_body(tc, d, score)
    _split_waits(nc)
    return nc


def get_nc():
    if "nc" not in _CACHE:
        _CACHE["nc"] = _build()
    return _CACHE["nc"]


def _make_in_maps(inputs):
    f = lambda a: np.ascontiguousarray(np.asarray(a, dtype=np.float32))
    x = f(inputs["x"])
    Tm = f(inputs["T"]).reshape(64, 250)
    Sa = np.zeros((125, 64), np.float32)
    Sb = np.zeros((125, 64), np.float32)
    for fk in range(125):
        Sa[fk, fk // 5] = 1.0
        Sb[fk, 25 + fk // 5] = 1.0
    TmS = Tm.reshape(64, 50, 5).sum(axis=2)
    TmSn = np.ascontiguousarray(-TmS)
    I50h2 = np.zeros((50, 128), np.float32)
    np.fill_diagonal(I50h2[:, 0:50], 0.5)
    np.fill_diagonal(I50h2[:, 64:114], 0.5)
    Ws = f(inputs["Ws"])
    bsf = np.array([[float(f(inputs["bs"]).reshape(-1)[0]) - float(Ws[64:].sum())]],
                   np.float32)
    CPF = np.zeros((128, 1015), np.float32)
    CPF[:, 0:256] = f(inputs["W1"])
    CPF[:, 256:384] = f(inputs["W2"])[0:128]
    CPF[:, 384:512] = f(inputs["W2"])[128:256]
    CPF[:, 512:576] = f(inputs["W3"])
    CPF[0:64, 576:640] = f(inputs["Wv"])
    CPF[0:64, 640:704] = f(inputs["Wo"])
    CPF[0:64, 704:954] = Tm
    CPF[0:64, 954:1004] = TmSn
    CPF[:, 1004] = f(inputs["b1"]).reshape(-1)[0:128]
    CPF[:, 1005] = f(inputs["b1"]).reshape(-1)[128:256]
    CPF[:, 1006] = f(inputs["b2"]).reshape(-1)
    CPF[:, 1007] = f(inputs["gamma"]).reshape(-1)
    CPF[:, 1008] = f(inputs["beta"]).reshape(-1)
    CPF[0:64, 1009] = f(inputs["b3"]).reshape(-1)
    CPF[0:64, 1010] = f(inputs["bv"]).reshape(-1)
    CPF[0:64, 1011] = f(inputs["bo"]).reshape(-1)
    CPF[0:64, 1012] = Ws[:64, 0]
    CPF[0:50, 1013] = Ws[64:, 0]
    CPF[0, 1014] = bsf[0, 0]
    CPB = np.zeros((125, 256), np.float32)
    CPB[:, 0:64] = Sa
    CPB[:, 64:128] = Sb
    CPB[0:50, 128:256] = I50h2
    common = {
        "CPF": CPF,
        "CPB": CPB.astype(ml_dtypes.bfloat16),
    }
    in_maps = []
    for c in range(NCORES):
        m = dict(common)
        m["xT"] = np.ascontiguousarray(np.roll(x, -JSH * c, axis=0).T)
        in_maps.append(m)
    return in_maps


def kernel(**inputs) -> np.ndarray:
    nc = get_nc()
    in_maps = _make_in_maps(inputs)
    res = run_bass_kernel_spmd(nc, in_maps, list(range(NCORES)))
    outs = [np.asarray(res.results[c]["score"]).reshape(JSH) for c in range(NCORES)]
    return np.concatenate(outs).astype(np.float32)


if __name__ == "__main__":
    print("building nc...")
    nc = get_nc()
    print("build OK")


# revision 49
# speedup vs baseline: 1.0451x; 1.0447x over previous
"""Trainium2 Bass kernel for nn_Discriminator (MLP + BN + attn + minibatch discrimination).

Strategy (8 NeuronCores, no collectives):
  - Shard the O(B^2) MBD block over the output index j: core d computes scores for
    batch rows [128d, 128d+128). SPMD programs are identical; the shard is selected
    by giving core d a batch-rolled copy of x (np.roll by -128d), so "my j's" are
    always local rows 0..127 while the i-sum still runs over the full batch.
  - Each core runs the full (tiny) MLP in transposed layout (features on partitions,
    batch on free axis), producing M^T [250, 1024] = (h @ T.reshape(64,250))^T.
  - Pairwise block per j: one dual-op tensor_scalar per 125-row fk-tile computes
    A = |M^T - M^T[:, j]| (bf16), a 0/1 selection-matrix matmul on the PE sums over
    k (5) into PSUM d = sum_k A [50 f, 1024 i] (two j's packed at PSUM partition
    offsets 0 and 64 via col-tiling), and one activation(Exp, scale=-1,
    accum_out=...) computes exp(-d) and the i-sum in a single ACT op.
  - score = Ws_h.T h + Ws_o.T o + (bs - sum(Ws_o)), the bias fold absorbing the
    reference's "-1" self-term removal.
"""

import numpy as np
from contextlib import ExitStack

import ml_dtypes
import concourse.bass as bass
import concourse.tile as tile
from concourse import mybir
from concourse.bass_utils import run_bass_kernel_spmd

F32 = mybir.dt.float32
BF16 = mybir.dt.bfloat16
AF = mybir.ActivationFunctionType
ALU = mybir.AluOpType
AX = mybir.AxisListType

B = 1024
IN_DIM = 128
NCORES = 8
JSH = B // NCORES          # 128 j's per core
NPAIR = JSH // 2           # 64 pairs of j's
FK = 250                   # 50 features x 5 kernels
FKH = 125                  # fk half-tile (f 0..24 | f 25..49)
NF = 50
BN_EPS = 1e-5

_CACHE: dict = {}


def _emit_body(tc, d, score_out):
    nc = tc.nc
    ctx = ExitStack()
    with ctx:
        consts = ctx.enter_context(tc.tile_pool(name="consts", bufs=1))
        mlp = ctx.enter_context(tc.tile_pool(name="mlp", bufs=1))
        small = ctx.enter_context(tc.tile_pool(name="small", bufs=1))

        # ---- load constants ----
        def cload(name, shape, dtype=F32):
            t = consts.tile(shape, dtype, tag=name)
            nc.sync.dma_start(t[:], d[name][:])
            return t

        # all f32 constants ride in one packed DMA; bf16 in a second
        CPF = consts.tile([128, 1015], F32, tag="CPF")
        nc.sync.dma_start(CPF[:], d["CPF"][:])
        CPB = consts.tile([125, 256], BF16, tag="CPB")
        nc.sync.dma_start(CPB[:], d["CPB"][:])
        W1 = CPF[:, 0:256]
        W2a = CPF[:, 256:384]
        W2b = CPF[:, 384:512]
        W3 = CPF[:, 512:576]
        Wv = CPF[0:64, 576:640]
        Wo = CPF[0:64, 640:704]
        Tm = CPF[0:64, 704:954]
        TmSn = CPF[0:64, 954:1004]
        b1a = CPF[:, 1004:1005]
        b1b = CPF[:, 1005:1006]
        b2 = CPF[:, 1006:1007]
        gamma = CPF[:, 1007:1008]
        beta = CPF[:, 1008:1009]
        b3 = CPF[0:64, 1009:1010]
        bv = CPF[0:64, 1010:1011]
        bo = CPF[0:64, 1011:1012]
        WsH = CPF[0:64, 1012:1013]
        WsO = CPF[0:50, 1013:1014]
        bsf = CPF[0:1, 1014:1015]
        Sa = CPB[:, 0:64]
        Sb = CPB[:, 64:128]
        I50h2 = CPB[0:50, 128:256]

        # ---- persistent activations (feature-major) ----
        xT = mlp.tile([128, B], F32, tag="xT")
        h1T = mlp.tile([128, 2 * B], F32, tag="h1T")      # [256,1024] as 2 M-tiles
        hbnT = mlp.tile([128, B], F32, tag="hbnT")
        h3T = mlp.tile([64, B], F32, tag="h3T")
        uT = mlp.tile([64, B], F32, tag="uT")
        hT = mlp.tile([64, B], F32, tag="hT")
        MTf = mlp.tile([125, 2 * B], F32, tag="MTf")      # [250,1024] as 2 fk-tiles
        MTb = mlp.tile([125, 2 * B], BF16, tag="MTb")
        OBUF = mlp.tile([128, NPAIR], F32, tag="OBUF")
        O50 = mlp.tile([50, NPAIR, 2], F32, tag="O50")
        SMTn = mlp.tile([50, B], F32, tag="SMTn")
        SMTnb = mlp.tile([50, B], BF16, tag="SMTnb")
        BIASP = mlp.tile([128, NPAIR], F32, tag="BIASP")

        def lrelu(dst, src):
            # dst = max(src, 0.2*src)
            nc.vector.scalar_tensor_tensor(
                out=dst, in0=src, scalar=0.2, in1=src, op0=ALU.mult, op1=ALU.max
            )

        with tc.tile_pool(name="ph1_psum", bufs=1, space=bass.MemorySpace.PSUM) as pp, \
             tc.tile_pool(name="ph1_sb", bufs=4) as sb:
            # ---- xT loaded directly (host pre-transposes x) ----
            nc.sync.dma_start(xT[:], d["xT"][:])

            # ---- h1T = lrelu(W1.T xT + b1) ----
            for mt, b1t in ((0, b1a), (1, b1b)):
                for c in range(2):
                    ps = pp.tile([128, 512], F32, tag="ps", bufs=2)
                    nc.tensor.matmul(ps[:], W1[:, 128 * mt:128 * (mt + 1)],
                                     xT[:, 512 * c:512 * (c + 1)], start=True, stop=True)
                    tt = sb.tile([128, 512], F32, tag="tt")
                    nc.scalar.activation(tt[:], ps[:], AF.Identity, bias=b1t, scale=1.0)
                    lrelu(h1T[:, B * mt + 512 * c: B * mt + 512 * (c + 1)], tt[:])

            # ---- h2 (kept in PSUM) + BN stats ----
            h2ps = []
            sums = small.tile([128, 4], F32, tag="sums")   # per-chunk sum, sumsq
            for c in range(2):
                ps = pp.tile([128, 512], F32, tag=f"h2ps{c}")
                for kt, W2t in ((0, W2a), (1, W2b)):
                    nc.tensor.matmul(ps[:], W2t[:],
                                     h1T[:, B * kt + 512 * c: B * kt + 512 * (c + 1)],
                                     start=(kt == 0), stop=(kt == 1))
                # bias b2 folds into BN shift below (h2+b2 then BN). Since BN
                # subtracts the batch mean, adding b2 cancels: (h+b2) - mean(h+b2)
                # = h - mean(h). Variance likewise unaffected. So skip b2 here.
                nc.vector.tensor_reduce(sums[:, c:c + 1], ps[:], axis=AX.X, op=ALU.add)
                sq = sb.tile([128, 512], F32, tag="sq")
                nc.scalar.activation(sq[:], ps[:], AF.Square, bias=0.0, scale=1.0,
                                     accum_out=sums[:, 2 + c:3 + c])
                h2ps.append(ps)

            # mu = (s0+s1)/1024 ; msq = (q0+q1)/1024 ; var = msq - mu^2
            mu = small.tile([128, 1], F32, tag="mu")
            nc.vector.scalar_tensor_tensor(out=mu[:], in0=sums[:, 0:1], scalar=1.0 / B,
                                           in1=sums[:, 1:2], op0=ALU.bypass, op1=ALU.add)
            nc.vector.tensor_scalar(out=mu[:], in0=mu[:], scalar1=1.0 / B, scalar2=None,
                                    op0=ALU.mult)
            msq = small.tile([128, 1], F32, tag="msq")
            nc.vector.scalar_tensor_tensor(out=msq[:], in0=sums[:, 2:3], scalar=1.0,
                                           in1=sums[:, 3:4], op0=ALU.bypass, op1=ALU.add)
            nc.vector.tensor_scalar(out=msq[:], in0=msq[:], scalar1=1.0 / B, scalar2=None,
                                    op0=ALU.mult)
            var = small.tile([128, 1], F32, tag="var")
            nc.vector.scalar_tensor_tensor(out=var[:], in0=mu[:], scalar=-1.0,
                                           in1=mu[:], op0=ALU.mult, op1=ALU.mult)
            nc.vector.tensor_tensor(out=var[:], in0=var[:], in1=msq[:], op=ALU.add)
            # invstd = exp(-0.5*ln(var+eps))  (avoids the banned Rsqrt and the sqrt table set)
            eps_t = small.tile([128, 1], F32, tag="eps")
            nc.vector.memset(eps_t[:], BN_EPS)
            lnv = small.tile([128, 1], F32, tag="lnv")
            nc.scalar.activation(lnv[:], var[:], AF.Ln, bias=eps_t[:], scale=1.0)
            invstd = small.tile([128, 1], F32, tag="invstd")
            nc.scalar.activation(invstd[:], lnv[:], AF.Exp, bias=0.0, scale=-0.5)
            # s = gamma*invstd ; bb = beta - mu*s  (+ b2 folded: cancels, see above)
            s = small.tile([128, 1], F32, tag="s")
            nc.vector.tensor_tensor(out=s[:], in0=invstd[:], in1=gamma[:], op=ALU.mult)
            bb = small.tile([128, 1], F32, tag="bb")
            nc.vector.scalar_tensor_tensor(out=bb[:], in0=mu[:], scalar=-1.0,
                                           in1=s[:], op0=ALU.mult, op1=ALU.mult)
            nc.vector.tensor_tensor(out=bb[:], in0=bb[:], in1=beta[:], op=ALU.add)

            # hbnT = lrelu(s*h2 + bb)
            for c in range(2):
                tt = sb.tile([128, 512], F32, tag="tt")
                nc.scalar.activation(tt[:], h2ps[c][:], AF.Identity, bias=bb[:, 0:1],
                                     scale=s[:, 0:1])
                lrelu(hbnT[:, 512 * c:512 * (c + 1)], tt[:])

            # ---- h3T = lrelu(W3.T hbnT + b3) ----
            for c in range(2):
                ps = pp.tile([64, 512], F32, tag="ps64", bufs=2)
                nc.tensor.matmul(ps[:], W3[:], hbnT[:, 512 * c:512 * (c + 1)],
                                 start=True, stop=True)
                tt = sb.tile([64, 512], F32, tag="tt64")
                nc.scalar.activation(tt[:], ps[:], AF.Identity, bias=b3, scale=1.0)
                lrelu(h3T[:, 512 * c:512 * (c + 1)], tt[:])

            # ---- uT = Wv.T h3T + bv ----
            for c in range(2):
                ps = pp.tile([64, 512], F32, tag="ps64", bufs=2)
                nc.tensor.matmul(ps[:], Wv[:], h3T[:, 512 * c:512 * (c + 1)],
                                 start=True, stop=True)
                nc.scalar.activation(uT[:, 512 * c:512 * (c + 1)], ps[:], AF.Identity,
                                     bias=bv, scale=1.0)

            # ---- hT = h3T + Wo.T uT + bo ----
            for c in range(2):
                ps = pp.tile([64, 512], F32, tag="ps64", bufs=2)
                nc.tensor.matmul(ps[:], Wo[:], uT[:, 512 * c:512 * (c + 1)],
                                 start=True, stop=True)
                nc.vector.scalar_tensor_tensor(
                    out=hT[:, 512 * c:512 * (c + 1)], in0=ps[:], scalar=bo,
                    in1=h3T[:, 512 * c:512 * (c + 1)], op0=ALU.add, op1=ALU.add)

            # ---- MT = Tm.T hT  ([250,1024] as 2 fk-tiles), f32 + bf16 copies ----
            for st in range(2):
                for c in range(2):
                    ps = pp.tile([125, 512], F32, tag="psm", bufs=1)
                    nc.tensor.matmul(ps[:], Tm[:, 125 * st:125 * (st + 1)],
                                     hT[:, 512 * c:512 * (c + 1)], start=True, stop=True)
                    sl = slice(B * st + 512 * c, B * st + 512 * (c + 1))
                    nc.vector.tensor_copy(MTf[:, sl], ps[:])
                    nc.scalar.activation(MTb[:, sl], ps[:], AF.Copy, bias=0.0, scale=1.0)

            # ---- SMTn[f, i] = -sum_k M[i, 5f+k]  (for the |d|=2relu(d)-d trick) ----
            for c in range(2):
                ps = pp.tile([50, 512], F32, tag="psm2", bufs=1)
                nc.tensor.matmul(ps[:], TmSn[:], hT[:, 512 * c:512 * (c + 1)],
                                 start=True, stop=True)
                nc.vector.tensor_copy(SMTn[:, 512 * c:512 * (c + 1)], ps[:])
                nc.scalar.activation(SMTnb[:, 512 * c:512 * (c + 1)], ps[:], AF.Copy,
                                     bias=0.0, scale=1.0)

            # per-pair exp bias rows: [0:50] <- SMTn col j1, [64:114] <- SMTn col j2
            nc.vector.memset(BIASP[:], 0.0)
            nc.vector.tensor_copy(BIASP[0:50, :], SMTn[:, 0:JSH].rearrange(
                "p (a b) -> p a b", b=2)[:, :, 0:1])
            nc.vector.tensor_copy(BIASP[64:114, :], SMTn[:, 0:JSH].rearrange(
                "p (a b) -> p a b", b=2)[:, :, 1:2])

        # ---- pairwise MBD block ----
        # d[f,i] for row j is sum_k |M_i - M_j| = 2*sum_k relu(M_i - M_j)
        #   - sum_k M_i + sum_k M_j.  PSUM accumulates P = SAp + 0.5*SMTn_i;
        # exp(-d) = Exp(-2*P + bias) with per-partition bias = SMTn[:, j].
        with tc.tile_pool(name="apool", bufs=16) as apool, \
             tc.tile_pool(name="dpool", bufs=4, space=bass.MemorySpace.PSUM) as dpool:
            for jp in range(NPAIR):
                j1, j2 = 2 * jp, 2 * jp + 1
                As = {}
                for (jj, col) in ((j1, 0), (j2, 64)):
                    for st in range(2):
                        A = apool.tile([125, B], BF16, tag=f"A{col}{st}")
                        nc.vector.tensor_scalar(
                            out=A[:], in0=MTb[:, B * st:B * (st + 1)],
                            scalar1=MTf[:, B * st + jj:B * st + jj + 1],
                            scalar2=0.0, op0=ALU.subtract, op1=ALU.max)
                        As[(col, st)] = A
                dps = dpool.tile([128, B], F32, tag="dps")
                # Sa/Sb: col-group-interleaved so adjacent MMs can run
                # concurrently in the array; the SMT correction rides ONE
                # full-width M=128 matmul (same rhs for both col groups)
                for st, S in ((0, Sa), (1, Sb)):
                    for c in range(2):
                        cs = slice(512 * c, 512 * (c + 1))
                        for col in (0, 64):
                            nc.tensor.matmul(dps[col:col + 64, cs], S[:],
                                             As[(col, st)][:, cs],
                                             start=(st == 0), stop=False,
                                             tile_position=(0, col),
                                             skip_group_check=True)
                for c in range(2):
                    cs = slice(512 * c, 512 * (c + 1))
                    nc.tensor.matmul(dps[0:128, cs], I50h2[:], SMTnb[:, cs],
                                     start=False, stop=True,
                                     skip_group_check=True)
                nc.scalar.activation(dps[0:114, :], dps[0:114, :], AF.Exp,
                                     bias=BIASP[0:114, jp:jp + 1], scale=-2.0,
                                     accum_out=OBUF[0:114, jp:jp + 1])

            # ---- o columns -> j-ordered [50, 128] ----
            nc.vector.tensor_copy(O50[:, :, 0:1], OBUF[0:50, :])
            nc.vector.tensor_copy(O50[:, :, 1:2], OBUF[64:114, :])

        # ---- score = WsH.T hT[:, :128] + WsO.T O + bsf (pairwise pools freed) ----
        with tc.tile_pool(name="spsum", bufs=1, space=bass.MemorySpace.PSUM) as sp:
            ssum = sp.tile([1, JSH], F32, tag="ssum")
            nc.tensor.matmul(ssum[:], WsH[:], hT[:, 0:JSH], start=True, stop=False)
            nc.tensor.matmul(ssum[:], WsO[:], O50[:, :, :], start=False, stop=True)
            sc = small.tile([1, JSH], F32, tag="sc")
            nc.scalar.activation(sc[:], ssum[:], AF.Identity, bias=bsf[0:1, 0:1],
                                 scale=1.0)
            nc.gpsimd.dma_start(score_out[:], sc[:])


def _split_waits(nc):
    """Hoist excess semaphore waits onto single-wait engine nops.

    This walrus build's codegen rejects instructions whose ISA struct carries
    more than one sync-wait ("Too many sync wait commands", e.g. the
    self-loading fp32 LDW+MM path). Engine instruction streams execute in
    order, so moving all waits of an instruction onto nop instructions spliced
    immediately before it (one wait per nop, same engine) is semantically
    identical. DMA instructions are left untouched (their waits ride the DGE
    descriptor, not the engine stream) and are asserted to have <=1 wait.
    """
    from concourse import mybir as mb
    DMA_TYPES = (mb.InstDMACopy, mb.InstDMA, mb.InstTriggeredCopy) \
        if hasattr(mb, "InstTriggeredCopy") else (mb.InstDMACopy, mb.InstDMA)
    for fn in nc.m.functions:
        for bb in fn.blocks:
            insts = list(bb.instructions)
            out = []
            for inst in insts:
                si = inst.sync_info
                waits = list(si.on_wait) if si is not None else []
                if len(waits) > 1:
                    if isinstance(inst, DMA_TYPES):
                        raise AssertionError(
                            f"DMA instruction {inst.name} has {len(waits)} waits; "
                            "cannot split safely — restructure the kernel")
                    for w in waits:
                        nop = mb.InstNoOp(
                            name=nc.get_next_instruction_name(),
                            ins=[], outs=[])
                        nop.engine = inst.engine
                        nop.sync_info = mb.SyncInfo(on_wait=[w], on_update=[])
                        nc.register_instruction(nop)
                        out.append(nop)
                    inst.sync_info = mb.SyncInfo(
                        on_wait=[], on_update=list(si.on_update))
                out.append(inst)
            bb.instructions = out


def _build():
    nc = bass.Bass("TRN2", target_bir_lowering=False, debug=False,
                   num_devices=NCORES)
    d = {}

    def din(name, shape, dtype=F32):
        d[name] = nc.dram_tensor(name, shape, dtype, kind="ExternalInput").ap()

    din("xT", [IN_DIM, B])
    din("CPF", [128, 1015])
    din("CPB", [125, 256], BF16)
    score = nc.dram_tensor("score", [1, JSH], F32, kind="ExternalOutput").ap()

    with tile.TileContext(nc) as tc:
        _emit_body(tc, d, score)
    _split_waits(nc)
    return nc


def get_nc():
    if "nc" not in _CACHE:
        _CACHE["nc"] = _build()
    return _CACHE["nc"]


def _make_in_maps(inputs):
    f = lambda a: np.ascontiguousarray(np.asarray(a, dtype=np.float32))
    x = f(inputs["x"])
    Tm = f(inputs["T"]).reshape(64, 250)
    Sa = np.zeros((125, 64), np.float32)
    Sb = np.zeros((125, 64), np.float32)
    for fk in range(125):
        Sa[fk, fk // 5] = 1.0
        Sb[fk, 25 + fk // 5] = 1.0
    TmS = Tm.reshape(64, 50, 5).sum(axis=2)
    TmSn = np.ascontiguousarray(-TmS)
    I50h2 = np.zeros((50, 128), np.float32)
    np.fill_diagonal(I50h2[:, 0:50], 0.5)
    np.fill_diagonal(I50h2[:, 64:114], 0.5)
    Ws = f(inputs["Ws"])
    bsf = np.array([[float(f(inputs["bs"]).reshape(-1)[0]) - float(Ws[64:].sum())]],
                   np.float32)
    CPF = np.zeros((128, 1015), np.float32)
    CPF[:, 0:256] = f(inputs["W1"])
    CPF[:, 256:384] = f(inputs["W2"])[0:128]
    CPF[:, 384:512] = f(inputs["W2"])[128:256]
    CPF[:, 512:576] = f(inputs["W3"])
    CPF[0:64, 576:640] = f(inputs["Wv"])
    CPF[0:64, 640:704] = f(inputs["Wo"])
    CPF[0:64, 704:954] = Tm
    CPF[0:64, 954:1004] = TmSn
    CPF[:, 1004] = f(inputs["b1"]).reshape(-1)[0:128]
    CPF[:, 1005] = f(inputs["b1"]).reshape(-1)[128:256]
    CPF[:, 1006] = f(inputs["b2"]).reshape(-1)
    CPF[:, 1007] = f(inputs["gamma"]).reshape(-1)
    CPF[:, 1008] = f(inputs["beta"]).reshape(-1)
    CPF[0:64, 1009] = f(inputs["b3"]).reshape(-1)
    CPF[0:64, 1010] = f(inputs["bv"]).reshape(-1)
    CPF[0:64, 1011] = f(inputs["bo"]).reshape(-1)
    CPF[0:64, 1012] = Ws[:64, 0]
    CPF[0:50, 1013] = Ws[64:, 0]
    CPF[0, 1014] = bsf[0, 0]
    CPB = np.zeros((125, 256), np.float32)
    CPB[:, 0:64] = Sa
    CPB[:, 64:128] = Sb
    CPB[0:50, 128:256] = I50h2
    common = {
        "CPF": CPF,
        "CPB": CPB.astype(ml_dtypes.bfloat16),
    }
    in_maps = []
    for c in range(NCORES):
        m = dict(common)
        m["xT"] = np.ascontiguousarray(np.roll(x, -JSH * c, axis=0).T)
        in_maps.append(m)
    return in_maps


def kernel(**inputs) -> np.ndarray:
    nc = get_nc()
    in_maps = _make_in_maps(inputs)
    res = run_bass_kernel_spmd(nc, in_maps, list(range(NCORES)))
    outs = [np.asarray(res.results[c]["score"]).reshape(JSH) for c in range(NCORES)]
    return np.concatenate(outs).astype(np.float32)


if __name__ == "__main__":
    print("building nc...")
    nc = get_nc()
    print("build OK")
